# revision 17
# baseline (speedup 1.0000x reference)
"""Trainium2 Bass kernel for BinarizedLinear + BatchNorm (training-mode, affine=False).

Computes: y = BN(sign(x) @ sign(W).T + bias), BN over the batch axis with
biased variance. bias is ignored: BN subtracts the batch mean, which absorbs
any per-feature constant exactly.

Sharding: data-parallel over the batch (B/8 rows per core). Each core also
owns 1/8 of W's rows (one 512-feature chunk): it binarizes them to fp8,
transposes to K-major on-chip (DMA transpose), and an AllGather distributes
the packed transposed chunks to every core.

The matmul computes out^T (features on PSUM partitions, batch on the free
axis) with the W chunk stationary and x moving, in fp8 DoubleRow perf mode.
This layout makes BN cheap:
  - batch sums / sums-of-squares are free-axis reductions fused into the
    PSUM->SBUF copy via the scalar engine's accum_out (Copy and Square).
  - per-feature mean/istd live one-per-partition, so the final normalize is
    a single scalar-engine Identity op with per-partition scale/bias.
Batch stats cross cores via small per-chunk-pair AllReduces, pipelined behind
the next chunks' matmuls. Output is written as y^T in fp16; the host
transposes and casts (values are ~N(0,1), fp16 rounding ~5e-4 << tolerance).

Numerics: sign values (+-1) are exact in fp8, products are +-1 and
accumulation happens in fp32 PSUM, so the matmul is exact. Raw outputs are
sums of IN odd terms -> even integers with |out| <= IN, exactly representable
in fp16. Stats accumulate in fp32.
"""

import numpy as np

import concourse.bass as bass
import concourse.mybir as mybir
import concourse.tile as tile
from concourse import bacc
from concourse.bass_utils import run_bass_kernel_spmd

P = 128
BN_EPS = 1e-5

F32 = mybir.dt.float32
BF16 = mybir.dt.bfloat16
F16 = mybir.dt.float16
F8 = mybir.dt.float8e4


class Cfg:
    def __init__(self, B=8192, IN=4096, OUT=4096, n_cores=8, oc=512):
        assert OUT // oc == n_cores, "one output chunk per core"
        self.B, self.IN, self.OUT, self.n_cores = B, IN, OUT, n_cores
        self.B_SH = B // n_cores          # batch rows per core
        self.BT = self.B_SH // P          # batch tiles per core (8)
        self.KT = IN // P                 # contraction (K) tiles (32)
        self.OC = oc                      # output-feature chunk width (512)
        self.NOC = OUT // oc              # number of output chunks (== n_cores)
        self.S = oc // P                  # W row-tiles per chunk (4)
        self.WH = min(2048, IN)           # load half-width (free elems)
        self.NH = IN // self.WH           # loads per row-tile (2)
        self.KH = self.WH // P            # K tiles per load (16)
        self.NSW = self.B_SH // 512       # batch swaths of 512 (2)
        self.SWT = 512 // P               # batch tiles per swath (4)


def build_program(cfg: Cfg, reps: int = 1, dbg: bool = False):
    """Build the SPMD Bass program (same NEFF on every core)."""
    nc = bacc.Bacc(
        "TRN2",
        target_bir_lowering=False,
        debug=False,
        enable_asserts=False,
        num_devices=cfg.n_cores,
    )

    x_in = nc.dram_tensor("x_shard", [cfg.B_SH, cfg.IN], F32, kind="ExternalInput")
    w_in = nc.dram_tensor("w_slice", [cfg.OC, cfg.IN], F32, kind="ExternalInput")
    # y^T: features x local batch, fp16 (host transposes + casts)
    y_out = nc.dram_tensor("y", [cfg.OUT, cfg.B_SH], F16, kind="ExternalOutput")

    coll_space = "Shared" if cfg.n_cores > 4 else "Local"
    # packed K-major fp8 transposed W chunk: [p, (k, s, o')] layout
    CHW = cfg.KT * cfg.S * P  # packed columns per chunk (16384)
    ag_in = nc.dram_tensor("ag_in", [P, CHW], F8, kind="Internal")
    ag_out = nc.dram_tensor(
        "ag_out", [cfg.n_cores * P, CHW], F8, kind="Internal",
        addr_space=coll_space,
    )
    # per-chunk stats: [oc, partition(o'), (sum s=0..3, sumsq s=0..3)]
    st_in = nc.dram_tensor("stats_in", [cfg.NOC, P, 2 * cfg.S], F32, kind="Internal")
    st_out = nc.dram_tensor(
        "stats_out", [cfg.NOC, P, 2 * cfg.S], F32, kind="Internal",
        addr_space=coll_space,
    )

    groups = [list(range(cfg.n_cores))]

    dbg_t = None
    if dbg:
        dbg_t = dict(
            d_xt=nc.dram_tensor("d_xt", [P, cfg.BT, cfg.KT, P], F8,
                                kind="ExternalOutput"),
            d_w8=nc.dram_tensor("d_w8", [P, cfg.KT, cfg.S, P], F8,
                                kind="ExternalOutput"),
            d_raw=nc.dram_tensor("d_raw", [P, cfg.NOC, cfg.S, cfg.B_SH], F16,
                                 kind="ExternalOutput"),
            d_st=nc.dram_tensor("d_st", [cfg.NOC, P, 2 * cfg.S], F32,
                                kind="ExternalOutput"),
            d_g8=nc.dram_tensor("d_g8", [cfg.NOC, P, 2 * cfg.S], F32,
                                kind="ExternalOutput"),
        )

    with tile.TileContext(nc) as tc:
        with (
            tc.tile_pool(name="const", bufs=1) as const,
            tc.tile_pool(name="xt", bufs=1) as xtp,
            tc.tile_pool(name="raw", bufs=1) as rawp,
            tc.tile_pool(name="bwt", bufs=2) as bwtp,
            tc.tile_pool(name="stage", bufs=3) as stagep,
            tc.tile_pool(name="bin", bufs=2) as binp,
            tc.tile_pool(name="tmp", bufs=3) as tmpp,
            tc.tile_pool(name="sq", bufs=2) as sqp,
            tc.tile_pool(name="stt", bufs=3) as sttp,
            tc.tile_pool(name="nrm", bufs=3) as nrmp,
            tc.tile_pool(name="psm", bufs=6, space="PSUM") as psm,
        ):
            eps_t = const.tile([P, 1], F32, tag="eps")
            nc.vector.memset(eps_t[:], float(BN_EPS))

            pools = dict(
                xtp=xtp, rawp=rawp, bwtp=bwtp, stagep=stagep, binp=binp,
                tmpp=tmpp, sqp=sqp, sttp=sttp, nrmp=nrmp, psm=psm,
            )
            consts = dict(eps_t=eps_t)
            tensors = dict(
                x_in=x_in, w_in=w_in, y_out=y_out,
                ag_in=ag_in, ag_out=ag_out, st_in=st_in, st_out=st_out,
            )
            for _rep in range(reps):
                _emit_once(nc, tc, cfg, groups, tensors, pools, consts,
                           dbg_t if _rep == 0 else None)

    nc.compile()
    return nc


def _emit_once(nc, tc, cfg, groups, T, pools, C, dbg_t=None):
    xtp, rawp, bwtp = pools["xtp"], pools["rawp"], pools["bwtp"]
    stagep, binp, tmpp = pools["stagep"], pools["binp"], pools["tmpp"]
    sqp, sttp, nrmp, psm = pools["sqp"], pools["sttp"], pools["nrmp"], pools["psm"]
    eps_t = C["eps_t"]
    x_in, w_in, y_out = T["x_in"], T["w_in"], T["y_out"]
    ag_in, ag_out = T["ag_in"], T["ag_out"]
    st_in, st_out = T["st_in"], T["st_out"]
    inv_b = 1.0 / float(cfg.B)
    npair = cfg.KT // 2
    AF = mybir.ActivationFunctionType

    # xt[p, bt, k, b'] = sign(x)[bt*128+b', k*128+p]  (fp8, K-major)
    xt = xtp.tile([P, cfg.BT, cfg.KT, P], F8, tag="xt")
    # rawT[p(o'), oc, s, b] = out^T in fp16 (exact: even ints <= IN)
    rawT = rawp.tile([P, cfg.NOC, cfg.S, cfg.B_SH], F16, tag="rawT")

    # ---- W slice prep: sign -> DMA-transpose -> fp8 K-major -> DRAM -> AG ----
    # w8[p, k, s, o'] = sign(W_slice)[s*128 + o', k*128 + p]
    w8 = xtp.tile([P, cfg.KT, cfg.S, P], F8, tag="slice8")
    for s in range(cfg.S):
        wfs = []
        for h in range(cfg.NH):
            wf = stagep.tile([P, cfg.WH], F32, tag="wstage")
            eng = nc.sync if (s + h) % 2 == 0 else nc.scalar
            eng.dma_start(
                wf[:],
                w_in.ap()[s * P:(s + 1) * P, h * cfg.WH:(h + 1) * cfg.WH],
            )
            wfs.append(wf)
        for h in range(cfg.NH):
            wb = binp.tile([P, cfg.WH], BF16, tag="wbin")
            nc.scalar.sign(wb[:], wfs[h][:])
            tmp = tmpp.tile([P, cfg.KH, P], BF16, tag="tmp")
            nc.sync.dma_start(tmp[:], wb[:], transpose=True)
            nc.vector.tensor_copy(
                w8[:, h * cfg.KH:(h + 1) * cfg.KH, s, :], tmp[:]
            )
    nc.sync.dma_start(
        ag_in.ap()[:, :], w8[:].rearrange("p a b c -> p (a b c)")
    )

    # ---- distribute packed transposed W chunks (cheap on-chip collective) ----
    nc.gpsimd.collective_compute(
        "AllGather",
        mybir.AluOpType.bypass,
        replica_groups=groups,
        ins=[ag_in.ap().opt()],
        outs=[ag_out.ap().opt()],
    )

    # ---- x prep: sign -> DMA-transpose -> fp8 (no PE involvement) ----
    for bt in range(cfg.BT):
        wfs = []
        for h in range(cfg.NH):
            wf = stagep.tile([P, cfg.WH], F32, tag="wstage")
            eng = nc.sync if (bt + h) % 2 == 0 else nc.scalar
            eng.dma_start(
                wf[:],
                x_in.ap()[bt * P:(bt + 1) * P, h * cfg.WH:(h + 1) * cfg.WH],
            )
            wfs.append(wf)
        for h in range(cfg.NH):
            xb = binp.tile([P, cfg.WH], BF16, tag="wbin")
            nc.scalar.sign(xb[:], wfs[h][:])
            tmp = tmpp.tile([P, cfg.KH, P], BF16, tag="tmp")
            nc.sync.dma_start(tmp[:], xb[:], transpose=True)
            nc.vector.tensor_copy(
                xt[:, bt, h * cfg.KH:(h + 1) * cfg.KH, :], tmp[:]
            )

    def w_fetch(oc):
        bwt = bwtp.tile([P, cfg.KT, cfg.S, P], F8, tag="bwt")
        nc.sync.dma_start(
            bwt[:].rearrange("p a b c -> p (a b c)"),
            ag_out.ap()[oc * P:(oc + 1) * P, :],
        )
        return bwt

    def matmuls(oc, bwt):
        # out^T: psum[o', b] per (s, bsw); W stationary (reused across bsw)
        sacc = sttp.tile([P, cfg.S, cfg.NSW], F32, tag="sacc")
        qacc = sttp.tile([P, cfg.S, cfg.NSW], F32, tag="qacc")
        for s in range(cfg.S):
            pss = []
            for bsw in range(cfg.NSW):
                ps = psm.tile([P, 512], F32, tag="mm")
                pss.append(ps)
            for i in range(npair):
                lhsT = bwt[:, 2 * i:2 * i + 2, s, :]
                for bsw in range(cfg.NSW):
                    rhs = xt[
                        :, bsw * cfg.SWT:(bsw + 1) * cfg.SWT,
                        2 * i:2 * i + 2, :,
                    ].rearrange("p t k b -> p k t b")
                    nc.tensor.matmul(
                        pss[bsw][:],
                        lhsT,
                        rhs,
                        start=(i == 0),
                        stop=(i == npair - 1),
                        perf_mode=mybir.MatmulPerfMode.DoubleRow,
                    )
            for bsw in range(cfg.NSW):
                # fused PSUM->fp16 copy + batch-sum, and square + batch-sumsq
                nc.scalar.activation(
                    rawT[:, oc, s, bsw * 512:(bsw + 1) * 512],
                    pss[bsw][:],
                    AF.Copy,
                    accum_out=sacc[:, s, bsw:bsw + 1],
                )
                sq = sqp.tile([P, 512], BF16, tag="sq")
                nc.scalar.activation(
                    sq[:],
                    pss[bsw][:],
                    AF.Square,
                    accum_out=qacc[:, s, bsw:bsw + 1],
                )
        # reduce batch swaths -> [p, s] halves of one [p, 2S] tile
        st8 = sttp.tile([P, 2 * cfg.S], F32, tag="st8")
        nc.vector.tensor_reduce(
            st8[:, 0:cfg.S], sacc[:], mybir.AxisListType.X, mybir.AluOpType.add
        )
        nc.vector.tensor_reduce(
            st8[:, cfg.S:2 * cfg.S], qacc[:], mybir.AxisListType.X,
            mybir.AluOpType.add,
        )
        nc.gpsimd.dma_start(st_in.ap()[oc, :, :], st8[:])
        if dbg_t is not None:
            nc.gpsimd.dma_start(dbg_t["d_st"].ap()[oc, :, :], st8[:])

    def ar_pair(oc0, n):
        nc.gpsimd.collective_compute(
            "AllReduce",
            mybir.AluOpType.add,
            replica_groups=groups,
            ins=[st_in.ap()[oc0:oc0 + n].opt()],
            outs=[st_out.ap()[oc0:oc0 + n].opt()],
        )

    def normalize(oc):
        g8 = sttp.tile([P, 2 * cfg.S], F32, tag="g8")
        nc.gpsimd.dma_start(g8[:], st_out.ap()[oc, :, :])
        if dbg_t is not None:
            nc.gpsimd.dma_start(dbg_t["d_g8"].ap()[oc, :, :], g8[:])
        # mean, E[x^2] in one scale
        nc.vector.tensor_scalar_mul(g8[:], g8[:], inv_b)
        m = g8[:, 0:cfg.S]
        e2 = g8[:, cfg.S:2 * cfg.S]
        var = sttp.tile([P, cfg.S], F32, tag="var")
        nc.vector.tensor_mul(out=var[:], in0=m, in1=m)
        nc.vector.tensor_sub(out=var[:], in0=e2, in1=var[:])
        std = sttp.tile([P, cfg.S], F32, tag="std")
        nc.scalar.activation(std[:], var[:], mybir.ActivationFunctionType.Sqrt,
                             bias=eps_t[:])
        istd = sttp.tile([P, cfg.S], F32, tag="istd")
        nc.vector.reciprocal(istd[:], std[:])
        shift = sttp.tile([P, cfg.S], F32, tag="shift")
        nc.vector.scalar_tensor_tensor(
            out=shift[:], in0=m, scalar=-1.0, in1=istd[:],
            op0=mybir.AluOpType.mult, op1=mybir.AluOpType.mult,
        )
        for s in range(cfg.S):
            yt = nrmp.tile([P, cfg.B_SH], F16, tag="yt")
            nc.scalar.activation(
                yt[:],
                rawT[:, oc, s, :],
                mybir.ActivationFunctionType.Identity,
                bias=shift[:, s:s + 1],
                scale=istd[:, s:s + 1],
            )
            eng = nc.sync if s % 2 == 0 else nc.scalar
            eng.dma_start(
                y_out.ap()[oc * cfg.OC + s * P:oc * cfg.OC + (s + 1) * P, :],
                yt[:],
            )

    # ---- software-pipelined chunk loop (prefetch 2 chunks) ----
    pre = {0: w_fetch(0)}
    if cfg.NOC > 1:
        pre[1] = w_fetch(1)
    for oc in range(cfg.NOC):
        bwt = pre.pop(oc)
        if oc + 2 < cfg.NOC:
            pre[oc + 2] = w_fetch(oc + 2)
        matmuls(oc, bwt)
        if oc % 2 == 1:
            ar_pair(oc - 1, 2)
        if oc >= 3 and oc % 2 == 1:
            for o2 in (oc - 3, oc - 2):
                normalize(o2)
    for o2 in range(cfg.NOC - 2, cfg.NOC):
        normalize(o2)

    if dbg_t is not None:
        nc.sync.dma_start(
            dbg_t["d_xt"].ap().opt(), xt[:].rearrange("p a b c -> p (a b c)")
        )
        nc.sync.dma_start(
            dbg_t["d_w8"].ap().opt(), w8[:].rearrange("p a b c -> p (a b c)")
        )
        nc.sync.dma_start(
            dbg_t["d_raw"].ap().opt(),
            rawT[:].rearrange("p a b c -> p (a b c)"),
        )


_CACHE = {}


def _get_program(reps: int = 1):
    if reps not in _CACHE:
        _CACHE[reps] = build_program(Cfg(), reps=reps)
    return _CACHE[reps]


def kernel(x, weight, bias=None):
    cfg = Cfg()
    x = np.asarray(x, dtype=np.float32)
    weight = np.asarray(weight, dtype=np.float32)
    assert x.shape == (cfg.B, cfg.IN) and weight.shape == (cfg.OUT, cfg.IN)

    nc = _get_program()
    in_maps = [
        {
            "x_shard": np.ascontiguousarray(x[c * cfg.B_SH:(c + 1) * cfg.B_SH]),
            "w_slice": np.ascontiguousarray(weight[c * cfg.OC:(c + 1) * cfg.OC]),
        }
        for c in range(cfg.n_cores)
    ]
    res = run_bass_kernel_spmd(nc, in_maps, core_ids=list(range(cfg.n_cores)))
    out = np.concatenate(
        [res.results[c]["y"].T.astype(np.float32) for c in range(cfg.n_cores)],
        axis=0,
    )
    return out


# revision 18
# speedup vs baseline: 12.2326x; 12.2326x over previous
"""Trainium2 Bass kernel for BinarizedLinear + BatchNorm (training-mode, affine=False).

Computes: y = BN(sign(x) @ sign(W).T + bias), BN over the batch axis with
biased variance. bias is ignored: BN subtracts the batch mean, which absorbs
any per-feature constant exactly.

Sharding: data-parallel over the batch (B/8 rows per core). Each core also
owns 1/8 of W's rows (one 512-feature chunk): it binarizes them to fp8,
transposes to K-major on-chip (DMA transpose), and an AllGather distributes
the packed transposed chunks to every core.

The matmul computes out^T (features on PSUM partitions, batch on the free
axis) with the W chunk stationary and x moving, in fp8 DoubleRow perf mode.
This layout makes BN cheap:
  - batch sums / sums-of-squares are free-axis reductions fused into the
    PSUM->SBUF copy via the scalar engine's accum_out (Copy and Square).
  - per-feature mean/istd live one-per-partition, so the final normalize is
    a single scalar-engine Identity op with per-partition scale/bias.
Batch stats cross cores via small per-chunk-pair AllReduces, pipelined behind
the next chunks' matmuls. Output is written as y^T in fp16; the host
transposes and casts (values are ~N(0,1), fp16 rounding ~5e-4 << tolerance).

Numerics: sign values (+-1) are exact in fp8, products are +-1 and
accumulation happens in fp32 PSUM, so the matmul is exact. Raw outputs are
sums of IN odd terms -> even integers with |out| <= IN, exactly representable
in fp16. Stats accumulate in fp32.
"""

import numpy as np

import concourse.bass as bass
import concourse.mybir as mybir
import concourse.tile as tile
from concourse import bacc
from concourse.bass_utils import run_bass_kernel_spmd

P = 128
BN_EPS = 1e-5

F32 = mybir.dt.float32
BF16 = mybir.dt.bfloat16
F16 = mybir.dt.float16
F8 = mybir.dt.float8e4


class Cfg:
    def __init__(self, B=8192, IN=4096, OUT=4096, n_cores=8, oc=512):
        assert OUT // oc == n_cores, "one output chunk per core"
        self.B, self.IN, self.OUT, self.n_cores = B, IN, OUT, n_cores
        self.B_SH = B // n_cores          # batch rows per core
        self.BT = self.B_SH // P          # batch tiles per core (8)
        self.KT = IN // P                 # contraction (K) tiles (32)
        self.OC = oc                      # output-feature chunk width (512)
        self.NOC = OUT // oc              # number of output chunks (== n_cores)
        self.S = oc // P                  # W row-tiles per chunk (4)
        self.WH = min(2048, IN)           # load half-width (free elems)
        self.NH = IN // self.WH           # loads per row-tile (2)
        self.KH = self.WH // P            # K tiles per load (16)
        self.NSW = self.B_SH // 512       # batch swaths of 512 (2)
        self.SWT = 512 // P               # batch tiles per swath (4)


def build_program(cfg: Cfg, reps: int = 1, dbg: bool = False):
    """Build the SPMD Bass program (same NEFF on every core)."""
    nc = bacc.Bacc(
        "TRN2",
        target_bir_lowering=False,
        debug=False,
        enable_asserts=False,
        num_devices=cfg.n_cores,
    )

    x_in = nc.dram_tensor("x_shard", [cfg.B_SH, cfg.IN], F32, kind="ExternalInput")
    w_in = nc.dram_tensor("w_slice", [cfg.OC, cfg.IN], F32, kind="ExternalInput")
    # y^T: features x local batch, fp16 (host transposes + casts)
    y_out = nc.dram_tensor("y", [cfg.OUT, cfg.B_SH], F16, kind="ExternalOutput")

    coll_space = "Shared" if cfg.n_cores > 4 else "Local"
    # packed K-major fp8 transposed W chunk: [p, (k, s, o')] layout
    CHW = cfg.KT * cfg.S * P  # packed columns per chunk (16384)
    ag_in = nc.dram_tensor("ag_in", [P, CHW], F8, kind="Internal")
    ag_out = nc.dram_tensor(
        "ag_out", [cfg.n_cores * P, CHW], F8, kind="Internal",
        addr_space=coll_space,
    )
    # per-chunk stats: [oc, partition(o'), (sum s=0..3, sumsq s=0..3)]
    st_in = nc.dram_tensor("stats_in", [cfg.NOC, P, 2 * cfg.S], F32, kind="Internal")
    st_out = nc.dram_tensor(
        "stats_out", [cfg.NOC, P, 2 * cfg.S], F32, kind="Internal",
        addr_space=coll_space,
    )

    groups = [list(range(cfg.n_cores))]

    dbg_t = None
    if dbg:
        dbg_t = dict(
            d_xt=nc.dram_tensor("d_xt", [P, cfg.BT, cfg.KT, P], F8,
                                kind="ExternalOutput"),
            d_w8=nc.dram_tensor("d_w8", [P, cfg.KT, cfg.S, P], F8,
                                kind="ExternalOutput"),
            d_raw=nc.dram_tensor("d_raw", [P, cfg.NOC, cfg.S, cfg.B_SH], F16,
                                 kind="ExternalOutput"),
            d_st=nc.dram_tensor("d_st", [cfg.NOC, P, 2 * cfg.S], F32,
                                kind="ExternalOutput"),
            d_g8=nc.dram_tensor("d_g8", [cfg.NOC, P, 2 * cfg.S], F32,
                                kind="ExternalOutput"),
        )

    with tile.TileContext(nc) as tc:
        with (
            tc.tile_pool(name="const", bufs=1) as const,
            tc.tile_pool(name="xt", bufs=1) as xtp,
            tc.tile_pool(name="raw", bufs=1) as rawp,
            tc.tile_pool(name="bwt", bufs=2) as bwtp,
            tc.tile_pool(name="stage", bufs=3) as stagep,
            tc.tile_pool(name="bin", bufs=2) as binp,
            tc.tile_pool(name="tmp", bufs=3) as tmpp,
            tc.tile_pool(name="sq", bufs=2) as sqp,
            tc.tile_pool(name="stt", bufs=3) as sttp,
            tc.tile_pool(name="nrm", bufs=3) as nrmp,
            tc.tile_pool(name="psm", bufs=6, space="PSUM") as psm,
        ):
            eps_t = const.tile([P, 1], F32, tag="eps")
            nc.vector.memset(eps_t[:], float(BN_EPS))

            pools = dict(
                xtp=xtp, rawp=rawp, bwtp=bwtp, stagep=stagep, binp=binp,
                tmpp=tmpp, sqp=sqp, sttp=sttp, nrmp=nrmp, psm=psm,
            )
            consts = dict(eps_t=eps_t)
            tensors = dict(
                x_in=x_in, w_in=w_in, y_out=y_out,
                ag_in=ag_in, ag_out=ag_out, st_in=st_in, st_out=st_out,
            )
            for _rep in range(reps):
                _emit_once(nc, tc, cfg, groups, tensors, pools, consts,
                           dbg_t if _rep == 0 else None)

    nc.compile()
    return nc


def _emit_once(nc, tc, cfg, groups, T, pools, C, dbg_t=None):
    xtp, rawp, bwtp = pools["xtp"], pools["rawp"], pools["bwtp"]
    stagep, binp, tmpp = pools["stagep"], pools["binp"], pools["tmpp"]
    sqp, sttp, nrmp, psm = pools["sqp"], pools["sttp"], pools["nrmp"], pools["psm"]
    eps_t = C["eps_t"]
    x_in, w_in, y_out = T["x_in"], T["w_in"], T["y_out"]
    ag_in, ag_out = T["ag_in"], T["ag_out"]
    st_in, st_out = T["st_in"], T["st_out"]
    inv_b = 1.0 / float(cfg.B)
    npair = cfg.KT // 2
    AF = mybir.ActivationFunctionType

    # xt[p, bt, k, b'] = sign(x)[bt*128+b', k*128+p]  (fp8, K-major)
    xt = xtp.tile([P, cfg.BT, cfg.KT, P], F8, tag="xt")
    # rawT[p(o'), oc, s, b] = out^T in fp16 (exact: even ints <= IN)
    rawT = rawp.tile([P, cfg.NOC, cfg.S, cfg.B_SH], F16, tag="rawT")

    # ---- W slice prep: sign -> DMA-transpose -> fp8 K-major -> DRAM -> AG ----
    # w8[p, k, s, o'] = sign(W_slice)[s*128 + o', k*128 + p]
    w8 = xtp.tile([P, cfg.KT, cfg.S, P], F8, tag="slice8")
    for s in range(cfg.S):
        wfs = []
        for h in range(cfg.NH):
            wf = stagep.tile([P, cfg.WH], F32, tag="wstage")
            eng = nc.sync if (s + h) % 2 == 0 else nc.scalar
            eng.dma_start(
                wf[:],
                w_in.ap()[s * P:(s + 1) * P, h * cfg.WH:(h + 1) * cfg.WH],
            )
            wfs.append(wf)
        for h in range(cfg.NH):
            wb = binp.tile([P, cfg.WH], BF16, tag="wbin")
            nc.scalar.sign(wb[:], wfs[h][:])
            tmp = tmpp.tile([P, cfg.KH, P], BF16, tag="tmp")
            nc.sync.dma_start(tmp[:], wb[:], transpose=True)
            nc.vector.tensor_copy(
                w8[:, h * cfg.KH:(h + 1) * cfg.KH, s, :], tmp[:]
            )
    nc.sync.dma_start(
        ag_in.ap()[:, :], w8[:].rearrange("p a b c -> p (a b c)")
    )

    # ---- distribute packed transposed W chunks (cheap on-chip collective) ----
    nc.gpsimd.collective_compute(
        "AllGather",
        mybir.AluOpType.bypass,
        replica_groups=groups,
        ins=[ag_in.ap().opt()],
        outs=[ag_out.ap().opt()],
    )

    # ---- x prep: sign -> DMA-transpose -> fp8 (no PE involvement) ----
    for bt in range(cfg.BT):
        wfs = []
        for h in range(cfg.NH):
            wf = stagep.tile([P, cfg.WH], F32, tag="wstage")
            eng = nc.sync if (bt + h) % 2 == 0 else nc.scalar
            eng.dma_start(
                wf[:],
                x_in.ap()[bt * P:(bt + 1) * P, h * cfg.WH:(h + 1) * cfg.WH],
            )
            wfs.append(wf)
        for h in range(cfg.NH):
            xb = binp.tile([P, cfg.WH], BF16, tag="wbin")
            nc.scalar.sign(xb[:], wfs[h][:])
            tmp = tmpp.tile([P, cfg.KH, P], BF16, tag="tmp")
            nc.sync.dma_start(tmp[:], xb[:], transpose=True)
            nc.vector.tensor_copy(
                xt[:, bt, h * cfg.KH:(h + 1) * cfg.KH, :], tmp[:]
            )

    def w_fetch(oc):
        bwt = bwtp.tile([P, cfg.KT, cfg.S, P], F8, tag="bwt")
        nc.gpsimd.dma_start(
            bwt[:].rearrange("p a b c -> p (a b c)"),
            ag_out.ap()[oc * P:(oc + 1) * P, :],
        )
        return bwt

    def matmuls(oc, bwt):
        # out^T: psum[o', b] per (s, bsw); W stationary (reused across bsw)
        sacc = sttp.tile([P, cfg.S, cfg.NSW], F32, tag="sacc")
        qacc = sttp.tile([P, cfg.S, cfg.NSW], F32, tag="qacc")
        for s in range(cfg.S):
            pss = []
            for bsw in range(cfg.NSW):
                ps = psm.tile([P, 512], F32, tag="mm")
                pss.append(ps)
            for i in range(npair):
                lhsT = bwt[:, 2 * i:2 * i + 2, s, :]
                for bsw in range(cfg.NSW):
                    rhs = xt[
                        :, bsw * cfg.SWT:(bsw + 1) * cfg.SWT,
                        2 * i:2 * i + 2, :,
                    ].rearrange("p t k b -> p k t b")
                    nc.tensor.matmul(
                        pss[bsw][:],
                        lhsT,
                        rhs,
                        start=(i == 0),
                        stop=(i == npair - 1),
                        perf_mode=mybir.MatmulPerfMode.DoubleRow,
                    )
            for bsw in range(cfg.NSW):
                # fused PSUM->fp16 copy + batch-sum, and square + batch-sumsq
                nc.scalar.activation(
                    rawT[:, oc, s, bsw * 512:(bsw + 1) * 512],
                    pss[bsw][:],
                    AF.Copy,
                    accum_out=sacc[:, s, bsw:bsw + 1],
                )
                sq = sqp.tile([P, 512], BF16, tag="sq")
                nc.scalar.activation(
                    sq[:],
                    pss[bsw][:],
                    AF.Square,
                    accum_out=qacc[:, s, bsw:bsw + 1],
                )
        # reduce batch swaths -> [p, s] halves of one [p, 2S] tile
        st8 = sttp.tile([P, 2 * cfg.S], F32, tag="st8")
        nc.vector.tensor_reduce(
            st8[:, 0:cfg.S], sacc[:], mybir.AxisListType.X, mybir.AluOpType.add
        )
        nc.vector.tensor_reduce(
            st8[:, cfg.S:2 * cfg.S], qacc[:], mybir.AxisListType.X,
            mybir.AluOpType.add,
        )
        nc.gpsimd.dma_start(st_in.ap()[oc, :, :], st8[:])
        if dbg_t is not None:
            nc.gpsimd.dma_start(dbg_t["d_st"].ap()[oc, :, :], st8[:])

    def ar_pair(oc0, n):
        nc.gpsimd.collective_compute(
            "AllReduce",
            mybir.AluOpType.add,
            replica_groups=groups,
            ins=[st_in.ap()[oc0:oc0 + n].opt()],
            outs=[st_out.ap()[oc0:oc0 + n].opt()],
        )

    def normalize(oc):
        g8 = sttp.tile([P, 2 * cfg.S], F32, tag="g8")
        nc.gpsimd.dma_start(g8[:], st_out.ap()[oc, :, :])
        if dbg_t is not None:
            nc.gpsimd.dma_start(dbg_t["d_g8"].ap()[oc, :, :], g8[:])
        # mean, E[x^2] in one scale
        nc.vector.tensor_scalar_mul(g8[:], g8[:], inv_b)
        m = g8[:, 0:cfg.S]
        e2 = g8[:, cfg.S:2 * cfg.S]
        var = sttp.tile([P, cfg.S], F32, tag="var")
        nc.vector.tensor_mul(out=var[:], in0=m, in1=m)
        nc.vector.tensor_sub(out=var[:], in0=e2, in1=var[:])
        std = sttp.tile([P, cfg.S], F32, tag="std")
        nc.scalar.activation(std[:], var[:], mybir.ActivationFunctionType.Sqrt,
                             bias=eps_t[:])
        istd = sttp.tile([P, cfg.S], F32, tag="istd")
        nc.vector.reciprocal(istd[:], std[:])
        shift = sttp.tile([P, cfg.S], F32, tag="shift")
        nc.vector.scalar_tensor_tensor(
            out=shift[:], in0=m, scalar=-1.0, in1=istd[:],
            op0=mybir.AluOpType.mult, op1=mybir.AluOpType.mult,
        )
        for s in range(cfg.S):
            yt = nrmp.tile([P, cfg.B_SH], F16, tag="yt")
            nc.scalar.activation(
                yt[:],
                rawT[:, oc, s, :],
                mybir.ActivationFunctionType.Identity,
                bias=shift[:, s:s + 1],
                scale=istd[:, s:s + 1],
            )
            eng = nc.sync if s % 2 == 0 else nc.scalar
            eng.dma_start(
                y_out.ap()[oc * cfg.OC + s * P:oc * cfg.OC + (s + 1) * P, :],
                yt[:],
            )

    # ---- software-pipelined chunk loop (prefetch 2 chunks) ----
    pre = {0: w_fetch(0)}
    if cfg.NOC > 1:
        pre[1] = w_fetch(1)
    for oc in range(cfg.NOC):
        bwt = pre.pop(oc)
        if oc + 2 < cfg.NOC:
            pre[oc + 2] = w_fetch(oc + 2)
        matmuls(oc, bwt)
        if oc % 2 == 1:
            ar_pair(oc - 1, 2)
        if oc >= 3 and oc % 2 == 1:
            for o2 in (oc - 3, oc - 2):
                normalize(o2)
    for o2 in range(cfg.NOC - 2, cfg.NOC):
        normalize(o2)

    if dbg_t is not None:
        nc.sync.dma_start(
            dbg_t["d_xt"].ap().opt(), xt[:].rearrange("p a b c -> p (a b c)")
        )
        nc.sync.dma_start(
            dbg_t["d_w8"].ap().opt(), w8[:].rearrange("p a b c -> p (a b c)")
        )
        nc.sync.dma_start(
            dbg_t["d_raw"].ap().opt(),
            rawT[:].rearrange("p a b c -> p (a b c)"),
        )


_CACHE = {}


def _get_program(reps: int = 1):
    if reps not in _CACHE:
        _CACHE[reps] = build_program(Cfg(), reps=reps)
    return _CACHE[reps]


def kernel(x, weight, bias=None):
    cfg = Cfg()
    x = np.asarray(x, dtype=np.float32)
    weight = np.asarray(weight, dtype=np.float32)
    assert x.shape == (cfg.B, cfg.IN) and weight.shape == (cfg.OUT, cfg.IN)

    nc = _get_program()
    in_maps = [
        {
            "x_shard": np.ascontiguousarray(x[c * cfg.B_SH:(c + 1) * cfg.B_SH]),
            "w_slice": np.ascontiguousarray(weight[c * cfg.OC:(c + 1) * cfg.OC]),
        }
        for c in range(cfg.n_cores)
    ]
    res = run_bass_kernel_spmd(nc, in_maps, core_ids=list(range(cfg.n_cores)))
    out = np.concatenate(
        [res.results[c]["y"].T.astype(np.float32) for c in range(cfg.n_cores)],
        axis=0,
    )
    return out
